# revision 57
# baseline (speedup 1.0000x reference)
"""GRU-residual trajectory kernel for Trainium2 (8 NeuronCores, data-parallel).

Reference semantics (PyTorch GRUCell math), 2048 sequential steps:
    h' = (1-u) * n + u * h
    r  = sigmoid(W_ih_r z + b_ih_r + W_hh_r h + b_hh_r)   (same for u)
    n  = tanh(W_ih_n z + b_ih_n + r * (W_hh_n h + b_hh_n))
    z' = z + dt * (W_head h' + b_head)
Output traj = [z0, z1, ..., z_steps] per batch row.

Design (per core, batch shard Bc=2048, 4 chunks of 512 cols; all matmuls
bf16 at 1 cyc/row = 213 ns):
  State xb [68, Bc] bf16: rows 0-63 h, 64-66 z-image, 67 ones. The exact
  z is NOT kept in SBUF: it accumulates in persistent PSUM as
  Zacc = z_t - z0 via start=False dz matmuls (PSUM adds are exact fp32);
  z0 stays in SBUF fp32. Lazy-z weight fold keeps the z path off the
  recurrence spine: gates at step t contract [h_t; z_{t-1}; 1] with
  W'_gh = W_gh + dt*W_gz*W_head, b' = b + dt*W_gz*b_head, and the z-image
  is initialised to z0 - dt*b_head.

  Per chunk and step:
    g   = w1^T xb              -> [u'-preact ; r-preact] PSUM (u' negated
                                  so sigmoid gives 1-u directly)
    s   = sigmoid(g)            ACT -> bf16 SBUF, u' rows 0:64, r 64:128
    bank= w23h^T xb (h_n)      -> one half of a pair-shared PSUM bank
    bank= s_r * bank            DVE in place (only DVE may touch PSUM)
    bank+= w_in^T xb (i_n)      PE accumulate (start=False) - replaces a
                                  separate elementwise add
    n   = tanh(bank half)       ACT per chunk, bridges to base 0
    d = n - h; d *= u'; h += d  Pool (GPSIMD), all-SBUF bf16 base 0
                                  (GPSIMD cannot access PSUM; SBUF-SBUF
                                  ops must share base partitions)
    zb  = z0 + Zacc = z_t       DVE, bf16 out -> xb z rows (precedes this
                                  step's dz accumulate in program order)
    Zacc += w5^T xb             dz accumulate (reads the new h + ones row)
    DMA zs[t] <- zb             bf16; block t = z_t; epilogue adds block
                                  `steps`; host converts bf16 -> fp32 and
                                  prepends z0.
  Engine busy/step (cost model): DVE 5.3us, Pool 5.1, ACT 4.9, PE 3.4,
  SP 2.0; 64-step unrolled hardware loop. Cost-model estimate 13.7 ms
  for 2048 steps vs 19.4 ms for the fp32 baseline; rel err 8.5e-3
  (bf16 weights/state/gates/output) vs 2e-2 tolerance.
"""

import contextlib
import os
import sys

for p in ("/opt/trn_rl_repo",):
    if p not in sys.path:
        sys.path.insert(0, p)

import numpy as np
import ml_dtypes

import concourse.bacc as bacc
import concourse.bass as bass
import concourse.mybir as mybir
from concourse.tile import TileContext
from concourse.bass_utils import run_bass_kernel_spmd

N_CORES = 8
B_FULL = 16384
BC = B_FULL // N_CORES  # 2048 per core
D = 3
H = 64
K = 68
ONE = 67
STEPS = 2048
CHUNK = 512
N_CHUNKS = BC // CHUNK
UNROLL = 64

F32 = mybir.dt.float32
BF16 = mybir.dt.bfloat16
SIG = mybir.ActivationFunctionType.Sigmoid
TANH = mybir.ActivationFunctionType.Tanh

_NC_CACHE = {}


def _zrows(par):
    return slice(64, 67)


def _build(steps: int):
    if steps in _NC_CACHE:
        return _NC_CACHE[steps]
    nc = bacc.Bacc(None, target_bir_lowering=False)

    xb0 = nc.dram_tensor("xb0", [K, BC], BF16, kind="ExternalInput")
    z0d = nc.dram_tensor("z0d", [D, BC], F32, kind="ExternalInput")
    w1_d = [
        [nc.dram_tensor(f"w1_{par}{cp}", [K, 2 * H], BF16, kind="ExternalInput")
         for cp in range(2)]
        for par in range(2)
    ]
    w_in_d = [
        nc.dram_tensor(f"win_{par}", [K, H], BF16, kind="ExternalInput")
        for par in range(2)
    ]
    w23h = nc.dram_tensor("w23h", [68, H], BF16, kind="ExternalInput")
    w5 = nc.dram_tensor("w5", [68, D], BF16, kind="ExternalInput")
    zs = nc.dram_tensor("zs", [(steps + 1) * D, BC], BF16, kind="ExternalOutput")

    with TileContext(nc) as tc:
        with (
            tc.tile_pool(name="state", bufs=1) as state_pool,
            tc.tile_pool(name="wpool", bufs=1) as wpool,
            tc.tile_pool(name="spool", bufs=8) as spool,
            tc.tile_pool(name="npool", bufs=6) as npool,
            tc.tile_pool(name="dpool", bufs=8) as dpool,
            tc.tile_pool(name="pg", bufs=3, space="PSUM") as pg,
            tc.tile_pool(name="pb", bufs=3, space="PSUM") as pb,
            tc.tile_pool(name="pzacc", bufs=1, space="PSUM") as pzacc,
        ):
            xb = state_pool.tile([K, BC], BF16)
            z0s = state_pool.tile([D, BC], F32, tag="z0s")
            w1_t = [[None, None], [None, None]]
            for par in range(2):
                for cp in range(2):
                    w1t = wpool.tile([K, 2 * H], BF16, tag=f"w1_{par}{cp}")
                    w1_t[par][cp] = w1t
                    nc.sync.dma_start(w1t[:], w1_d[par][cp][:])
            w_in_t = []
            for par in range(2):
                wint = wpool.tile([K, H], BF16, tag=f"win_{par}")
                w_in_t.append(wint)
                nc.sync.dma_start(wint[:], w_in_d[par][:])
            w23h_t = wpool.tile([68, H], BF16, tag="w23h")
            w5_t = wpool.tile([68, D], BF16, tag="w5")
            nc.sync.dma_start(w23h_t[:], w23h[:])
            nc.sync.dma_start(w5_t[:], w5[:])
            nc.sync.dma_start(xb[:], xb0[:])
            nc.sync.dma_start(z0s[:], z0d[:])

            zaccA = pzacc.tile([64 + D, CHUNK], F32, tag="zaccA")
            zaccB = pzacc.tile([D, CHUNK], F32, tag="zaccB")
            nc.vector.memset(zaccA[:], 0.0)
            nc.vector.memset(zaccB[:], 0.0)

            def zacc_ap(c):
                if c < 3:
                    return zaccA[32 * c : 32 * c + D, :]
                return zaccB[0:D, :]

            try:
                from concourse.hw_specs import get_activation_tables

                tabs = list(get_activation_tables(nc.m.arch).items())
                need = {SIG, TANH}
                set_id = next(i for i, (_, fns) in enumerate(tabs) if need <= fns)
            except Exception:
                set_id = 2
            nc.scalar.add_instruction(
                mybir.InstLoadActFuncSet(
                    name=nc.get_next_instruction_name(),
                    ins=[],
                    outs=[],
                    act_func_set_id=set_id,
                )
            )

            lo, hi = slice(0, H), slice(H, 2 * H)
            css = [slice(c * CHUNK, (c + 1) * CHUNK) for c in range(N_CHUNKS)]
            # u' always rows 0:H (SBUF-SBUF ops must share base partition),
            # r always rows H:2H (feeds the mixed SBUF/PSUM tmul, exempt).
            bsls = [hi, lo, hi, lo]  # h_n half within the pair bank

            def emit_step(t):
                par = t % 2          # z set written this step (holds z_t)
                gpar = 1 - par       # z set read by g1/i_n (holds z_{t-1})
                banks = [None, None]
                sss, gg, nts, ds = {}, {}, {}, {}
                for c in range(N_CHUNKS):
                    g = pg.tile([2 * H, CHUNK], F32, tag="g")
                    nc.tensor.matmul(
                        g[:], w1_t[gpar][0][:], xb[:, css[c]],
                        start=True, stop=True,
                    )
                    gg[c] = g
                for c in range(N_CHUNKS):
                    s = spool.tile([2 * H, CHUNK], BF16, tag="s")
                    nc.scalar.activation(s[:], gg[c][:], SIG)
                    sss[c] = s
                for p in range(2):
                    bank = pb.tile([2 * H, CHUNK], F32, tag="bank")
                    banks[p] = bank
                for c in range(N_CHUNKS):
                    nc.tensor.matmul(
                        banks[c // 2][bsls[c], :], w23h_t[:], xb[0:68, css[c]],
                        start=True, stop=True,
                    )
                for c in range(N_CHUNKS):
                    # r (rows H:2H) * h_n: SBUF x PSUM operands, any bases
                    nc.vector.tensor_mul(
                        banks[c // 2][bsls[c], :], sss[c][H : 2 * H, :],
                        banks[c // 2][bsls[c], :],
                    )
                for c in range(N_CHUNKS):
                    nc.tensor.matmul(
                        banks[c // 2][bsls[c], :], w_in_t[gpar][:],
                        xb[:, css[c]],
                        start=False, stop=True, skip_group_check=True,
                    )
                for c in range(N_CHUNKS):
                    # per-chunk tanh: ACT bridges the bank half to base 0
                    n_t = npool.tile([H, CHUNK], BF16, tag="n")
                    nc.scalar.activation(
                        n_t[:], banks[c // 2][bsls[c], :], TANH
                    )
                    nts[c] = n_t
                for c in range(N_CHUNKS):
                    # chunk-major tail: hadd(c) lands after 3 Pool ops so the
                    # chunk's next-step g1 can start early
                    d_t = dpool.tile([H, CHUNK], BF16, tag="d")
                    nc.gpsimd.tensor_sub(
                        d_t[:], nts[c][:], xb[0:H, css[c]]
                    )
                    nc.gpsimd.tensor_mul(d_t[:], d_t[:], sss[c][0:H, :])
                    nc.gpsimd.tensor_add(
                        xb[0:H, css[c]], xb[0:H, css[c]], d_t[:]
                    )
                    # zb = z0 + Zacc = z_t (precedes this step's dz accum);
                    # DVE (only PSUM-capable elementwise engine)
                    nc.vector.tensor_add(
                        xb[_zrows(par), css[c]], z0s[:, css[c]], zacc_ap(c)
                    )
                    nc.tensor.matmul(
                        zacc_ap(c), w5_t[:], xb[0:68, css[c]],
                        start=False, stop=True, skip_group_check=True,
                    )
                    nc.sync.dma_start(
                        zs[bass.ds(t * D, D), css[c]], xb[_zrows(par), css[c]]
                    )

            unroll = next(u for u in (UNROLL, 8, 4, 2, 1) if steps % u == 0)
            with tc.For_i(0, steps // unroll) as tu:
                for uu in range(unroll):
                    emit_step(tu * unroll + uu)

            # epilogue: final zb = z_steps -> zs block `steps`
            epar = steps % 2
            for c in (0, 1, 2, 3):
                nc.vector.tensor_add(
                    xb[_zrows(epar), css[c]], z0s[:, css[c]], zacc_ap(c)
                )
                nc.sync.dma_start(
                    zs[bass.ds(steps * D, D), css[c]], xb[_zrows(epar), css[c]]
                )

    nc.finalize()
    _NC_CACHE[steps] = nc
    return nc


def _pack_weights(dt, W_ih, W_hh, b_ih, b_hh, W_head, b_head):
    """Host-side packing: lazy-z fold + parity z-set variants (K=99)."""
    W_ih = np.asarray(W_ih, np.float64)
    W_hh = np.asarray(W_hh, np.float64)
    b_ih = np.asarray(b_ih, np.float64)
    b_hh = np.asarray(b_hh, np.float64)
    W_head = np.asarray(W_head, np.float64)
    b_head = np.asarray(b_head, np.float64)
    dt = float(dt)

    def fold(A, Hh, b):
        return (Hh + dt * A @ W_head).T, A.T, b + dt * A @ b_head

    A_r, H_r, b_r = W_ih[0:H], W_hh[0:H], b_ih[0:H] + b_hh[0:H]
    A_u, H_u, b_u = (
        W_ih[H : 2 * H], W_hh[H : 2 * H], b_ih[H : 2 * H] + b_hh[H : 2 * H]
    )

    bf = ml_dtypes.bfloat16

    def w1_variant(par):
        w = np.zeros((K, 2 * H), np.float64)
        zr = _zrows(0)
        hr, zrow, on = fold(-A_u, -H_u, -b_u)  # u' negated -> cols 0:H
        w[0:H, 0:H], w[zr, 0:H], w[ONE, 0:H] = hr, zrow, on
        hr, zrow, on = fold(A_r, H_r, b_r)  # r -> cols H:2H
        w[0:H, H : 2 * H], w[zr, H : 2 * H], w[ONE, H : 2 * H] = hr, zrow, on
        return w.astype(bf), w.astype(bf)

    w1 = [w1_variant(0), w1_variant(1)]  # [par][chunk%2]

    A_n, b_n = W_ih[2 * H : 3 * H], b_ih[2 * H : 3 * H]

    def win_variant(par):
        w = np.zeros((K, H), np.float64)
        hr, zrow, on = fold(A_n, np.zeros((H, H)), b_n)
        w[0:H, :], w[_zrows(0), :], w[ONE, :] = hr, zrow, on
        return w.astype(bf)

    w_in = [win_variant(0), win_variant(1)]

    w23h = np.zeros((68, H), np.float64)
    w23h[0:H, :] = W_hh[2 * H : 3 * H].T
    w23h[ONE, :] = b_hh[2 * H : 3 * H]

    w5 = np.zeros((68, D), np.float64)
    w5[0:H, :] = dt * W_head.T
    w5[ONE, :] = dt * b_head

    return w1, w_in, w23h.astype(bf), w5.astype(bf)


def kernel(z0, dt, steps, W_ih, W_hh, b_ih, b_hh, W_head, b_head):
    z0 = np.asarray(z0, np.float32)
    steps = int(steps)
    B, d = z0.shape
    assert (B, d) == (B_FULL, D)
    w1, w_in, w23h, w5 = _pack_weights(
        dt, W_ih, W_hh, b_ih, b_hh, W_head, b_head
    )
    dtf = float(dt)
    b_head64 = np.asarray(b_head, np.float64)

    nc = _build(steps)
    bf = ml_dtypes.bfloat16
    in_maps = []
    for c in range(N_CORES):
        z0c = z0[c * BC : (c + 1) * BC]  # [BC, 3]
        xb0 = np.zeros((K, BC), np.float64)
        # z_{-1} = z0 - dt*b_head in the (single) z-image rows
        xb0[_zrows(1), :] = z0c.T.astype(np.float64) - dtf * b_head64[:, None]
        xb0[ONE, :] = 1.0
        im = {
            "xb0": xb0.astype(bf),
            "z0d": np.ascontiguousarray(z0c.T),
            "w23h": w23h,
            "w5": w5,
        }
        for par in range(2):
            im[f"w1_{par}0"], im[f"w1_{par}1"] = w1[par]
            im[f"win_{par}"] = w_in[par]
        in_maps.append(im)
    res = run_bass_kernel_spmd(nc, in_maps, core_ids=list(range(N_CORES)))

    outs = []
    for c in range(N_CORES):
        zsb = res.results[c]["zs"].reshape(steps + 1, D, BC)
        traj = np.empty((BC, steps + 1, D), np.float32)
        traj[:, 0, :] = z0[c * BC : (c + 1) * BC]
        traj[:, 1:, :] = (
            np.asarray(zsb[1:]).astype(np.float32).transpose(2, 0, 1)
        )
        outs.append(traj)
    return np.concatenate(outs, axis=0)
